# revision 18
# baseline (speedup 1.0000x reference)
"""Trainium2 Bass kernel for a single-head causal attention module.

Problem (hardcoded): x [8, 2048, 1024] f32, W_Q/W_K/W_V [64, 1024] f32
    Q = x @ W_Q.T ; K = x @ W_K.T ; V = x @ W_V.T       (per batch)
    out = softmax(causal(Q @ K.T / sqrt(64))) @ V        -> [8, 2048, 64] f32

Sharding: batch dim across the 8 NeuronCores (data parallel, no collectives).

Per-core dataflow (all matmuls contract over the SBUF partition dim):
  1. PE-transpose x -> xT (d-major) since the QKV projections contract over d.
  2. One matmul chain per strip computes Q^T and K^T together (W_Q^T | W_K^T
     packed along the stationary free dim), output j-major which is exactly
     what the scores matmul needs to contract over j. Q^T/K^T strips are
     zero-padded to 128 partitions: a fp32 matmul with a 64-deep contraction
     streams at half rate, so padding the contraction doubles throughput.
  3. V^T likewise, then PE-transposed to V (s-major) with a ones column
     appended, so the P@V matmul also produces the softmax row-sums for free.
  4. Attention runs over 1024-wide q halves (pairs of 512 chunks): per key
     tile, scores land key-major (S^T) in a [128, 1024] PSUM half-strip, exp
     runs on ScalarE with the 1/sqrt(64) scale fused in (one call per half,
     amortizing ScalarE's fixed overhead), causal masking is an exact 0/1
     upper-triangular multiply on the diagonal block only (blocks above the
     diagonal are never computed), and P^T @ [V|1] accumulates O^T in PSUM.
  5. When a q chunk's last key tile retires, a PE transpose + reciprocal scale
     by the ones-column row-sum normalizes it into [128, 64] output tiles.
     The h=0 half (q < 1024) only needs strips 0-1, so it is emitted before
     strips 2-3 are built and overlaps their transposes/projections.
"""

import os

import numpy as np

import concourse.mybir as mybir
import concourse.tile as tile
from concourse import bacc
from concourse.bass_utils import run_bass_kernel_spmd
from concourse.masks import make_identity

B, S, D, J, P = 8, 2048, 1024, 64, 128
NCH = D // P  # 8 contraction chunks of 128
NSG = 4  # 512-wide s/q strips
SW = S // NSG  # 512
NT = S // P  # 16 key tiles
HW_ = 1024  # attention half-strip width
F32 = mybir.dt.float32

# Matmul input dtype: float32r streams 1 row/cycle (vs 4 for float32) with a
# 128-deep contraction; numerics measured at ~2e-4 max rel err end to end.
MM_DT = {
    "fp32": mybir.dt.float32,
    "fp32r": mybir.dt.float32r,
    "bf16": mybir.dt.bfloat16,
}[os.environ.get("ATTN_MM_DTYPE", "fp32r")]


def _build():
    nc = bacc.Bacc("TRN2", debug=False)
    x = nc.dram_tensor("x", [S, D], F32, kind="ExternalInput").ap()
    wqk = nc.dram_tensor("WQK", [D, P], F32, kind="ExternalInput").ap()
    wv = nc.dram_tensor("WV", [D, J], F32, kind="ExternalInput").ap()
    out = nc.dram_tensor("out", [S, J], F32, kind="ExternalOutput").ap()

    AF = mybir.ActivationFunctionType

    _fill = {}

    def zset(ap, val):  # memset that tolerates float32r tiles
        if ap.dtype != mybir.dt.float32r:
            nc.any.memset(ap, val)
            return
        # no engine memsets fp32r; copy-cast from a constant F32 tile instead
        src = _fill[val]
        nc.any.tensor_copy(ap, src[tuple(slice(0, d) for d in ap.shape)])

    with tile.TileContext(nc) as tc:
        from contextlib import ExitStack

        with ExitStack() as ctx:
            persist = ctx.enter_context(tc.tile_pool(name="persist", bufs=1))
            xsb_pool = ctx.enter_context(tc.tile_pool(name="xsb", bufs=2))
            wsb_pool = ctx.enter_context(tc.tile_pool(name="wsb", bufs=3))
            xsr_pool = ctx.enter_context(tc.tile_pool(name="xsr", bufs=2))
            pt_pool = ctx.enter_context(tc.tile_pool(name="ptp", bufs=3))
            otsb_pool = ctx.enter_context(tc.tile_pool(name="otsb", bufs=2))
            osb_pool = ctx.enter_context(tc.tile_pool(name="osb", bufs=3))
            rcp_pool = ctx.enter_context(tc.tile_pool(name="rcp", bufs=3))
            # PSUM budget (8 banks): six shared [128, 512] slots (transposes,
            # projections, scores, finalize transposes) + 2 O^T accumulators.
            ps = ctx.enter_context(tc.tile_pool(name="ps", bufs=6, space="PSUM"))
            pssc = ctx.enter_context(tc.tile_pool(name="pssc", bufs=1, space="PSUM"))

            for val in (0.0, 1.0):
                ft = persist.tile([P, SW], F32, tag=f"fill{int(val)}", name=f"fill{int(val)}")
                nc.gpsimd.memset(ft, val)
                _fill[val] = ft

            ident = persist.tile([P, P], F32, tag="ident")
            make_identity(nc, ident)
            if MM_DT != F32:
                identm = persist.tile([P, P], MM_DT, tag="identm")
                nc.vector.tensor_copy(identm, ident)
            else:
                identm = ident
            # triu[p, f] = 1.0 iff f >= p  (valid: q_local >= k_local)
            triu = persist.tile([P, P], F32, tag="triu")
            nc.gpsimd.memset(triu, 1.0)
            nc.gpsimd.affine_select(
                out=triu,
                in_=triu,
                compare_op=mybir.AluOpType.is_ge,
                fill=0.0,
                base=0,
                pattern=[[1, P]],
                channel_multiplier=-1,
            )

            x_r = x.rearrange("(t p) d -> p t d", p=P)  # [128, 16, 1024]
            xs_pre = xsb_pool.tile([P, 4, D], F32, tag="xs", name="xs_pre")
            nc.sync.dma_start(xs_pre[:, 0:1, :], x_r[:, 0:1, :])
            nc.sync.dma_start(xs_pre[:, 1:2, :], x_r[:, 1:2, :])
            # W_Q^T | W_K^T packed along the stationary free dim (host
            # pre-transposes the tiny weights at model-load time); W_V^T alone.
            wqk_t = persist.tile([P, NCH, P], MM_DT, tag="wqkt")
            wv_t = persist.tile([P, NCH, J], MM_DT, tag="wvt")
            wqk_f = wsb_pool.tile([P, NCH, P], F32, tag="wqkf", name="wqk_f")
            wv_f = wsb_pool.tile([P, NCH, J], F32, tag="wvf", name="wv_f")
            nc.sync.dma_start(wqk_f, wqk.rearrange("(c p) m -> p c m", p=P))
            nc.sync.dma_start(wv_f, wv.rearrange("(c p) m -> p c m", p=P))
            if MM_DT != F32:
                nc.any.tensor_copy(wqk_t, wqk_f)
                nc.any.tensor_copy(wv_t, wv_f)
            else:
                wqk_t, wv_t = wqk_f, wv_f

            nc.sync.dma_start(xs_pre[:, 2:4, :], x_r[:, 2:4, :])

            xt_s = [
                persist.tile([P, NCH, SW], MM_DT, tag=f"xt{g}", name=f"xt{g}")
                for g in range(NSG)
            ]
            qt_s = [
                persist.tile([P, SW], MM_DT, tag=f"qt{g}", name=f"qt{g}")
                for g in range(NSG)
            ]
            kt_s = [
                persist.tile([P, SW], MM_DT, tag=f"kt{g}", name=f"kt{g}")
                for g in range(NSG)
            ]
            vaug_s = [
                persist.tile([P, 4, 72], MM_DT, tag=f"va{g}", name=f"va{g}")
                for g in range(NSG)
            ]

            out_r = out.rearrange("(t p) j -> p t j", p=P)  # [128, 16, 64]

            def dma_strip(sg):
                xs = xsb_pool.tile([P, 4, D], F32, tag="xs", name="xs")
                for half in range(2):  # split DMA so transposes start earlier
                    nc.sync.dma_start(
                        xs[:, 2 * half : 2 * half + 2, :],
                        x_r[:, 4 * sg + 2 * half : 4 * sg + 2 * half + 2, :],
                    )
                return xs

            def build_strip(sg, xs):
                """Transpose strip sg, project Q^T|K^T, V^T, build V|1."""
                for dc in range(NCH):
                    pst = ps.tile([P, SW], F32, tag="ps512", name="pst")
                    for k in range(4):
                        nc.tensor.transpose(
                            pst[:, P * k : P * k + P],
                            xs[:, k, P * dc : P * dc + P],
                            ident,
                        )
                    nc.vector.tensor_copy(xt_s[sg][:, dc, :], pst)
                zset(kt_s[sg][J:P, :], 0.0)
                psqk = ps.tile([P, SW], F32, tag="ps512", name="psqk")
                for dc in range(NCH):
                    nc.tensor.matmul(
                        psqk,
                        wqk_t[:, dc, :],
                        xt_s[sg][:, dc, :],
                        start=(dc == 0),
                        stop=(dc == NCH - 1),
                    )
                nc.any.tensor_copy(qt_s[sg][0:J, :], psqk[0:J])
                nc.any.tensor_copy(kt_s[sg][0:J, :], psqk[J:P])
                psv = ps.tile([P, SW], F32, tag="ps512", name="psv")
                for dc in range(NCH):
                    nc.tensor.matmul(
                        psv[0:J],
                        wv_t[:, dc, :],
                        xt_s[sg][:, dc, :],
                        start=(dc == 0),
                        stop=(dc == NCH - 1),
                    )
                # V^T parks in the (zero-weighted) bottom half of the q strip
                nc.any.tensor_copy(qt_s[sg][J:P, :], psv[0:J])
                nc.any.tensor_copy(
                    vaug_s[sg][:, :, J : J + 1], _fill[1.0][:, 0:4].unsqueeze(-1)
                )
                for k in range(4):
                    psv2 = ps.tile([P, 72], MM_DT, tag="ps512", name="psv2")
                    nc.tensor.transpose(
                        psv2[:, 0:J],
                        qt_s[sg][J:P, P * k : P * k + P],
                        identm[J:P, J:P] if MM_DT != F32 else ident[J:P, J:P],
                    )
                    nc.any.tensor_copy(vaug_s[sg][:, k, 0:J], psv2[:, 0:J])

            def finalize_chunk(c, ot):
                """Normalize O^T chunk c and write [128, 64] output tiles."""
                otsb = otsb_pool.tile([J + 1, SW], F32, tag="otsb", name="otsb")
                nc.any.tensor_copy(otsb, ot)
                o = osb_pool.tile([P, 4, J], F32, tag="o", name="o")
                for k in range(4):
                    pso = ps.tile([P, 72], F32, tag="ps512", name="pso")
                    nc.tensor.transpose(
                        pso[:, 0 : J + 1],
                        otsb[:, P * k : P * k + P],
                        ident[0 : J + 1, 0 : J + 1],
                    )
                    rc = rcp_pool.tile([P, 1], F32, tag="rc", name="rc")
                    nc.vector.reciprocal(rc, pso[:, J : J + 1])
                    nc.vector.tensor_scalar_mul(out=o[:, k, :], in0=pso[:, 0:J], scalar1=rc)
                nc.sync.dma_start(out_r[:, 4 * c : 4 * c + 4, :], o)

            def attn_quarter(c):
                """Scores/softmax/PV for q chunk [512c, 512c+512)."""
                ot = pssc.tile([J + 1, SW], F32, tag=f"ot{c % 2}", name="ot")
                for t in range(4 * c + 4):
                    sgt, tl = t // 4, t % 4
                    co = max(0, P * t - SW * c)
                    pss = ps.tile([P, SW], F32, tag="ps512", name="pss")
                    nc.tensor.matmul(
                        pss[:, co:SW],
                        kt_s[sgt][:, P * tl : P * tl + P],
                        qt_s[c][:, co:SW],
                        start=True,
                        stop=True,
                    )
                    ptc = pt_pool.tile([P, SW], MM_DT, tag="ptc", name="ptc")
                    nc.scalar.activation(
                        ptc[:, co:SW], pss[:, co:SW], AF.Exp, scale=0.125
                    )
                    if t >= 4 * c:  # diagonal block
                        nc.vector.tensor_mul(
                            ptc[:, co : co + P], ptc[:, co : co + P], triu
                        )
                    nc.tensor.matmul(
                        ot[:, co:SW],
                        vaug_s[sgt][:, tl, 0 : J + 1],
                        ptc[:, co:SW],
                        start=(t == 0),
                        stop=(t == 4 * c + 3),
                    )
                finalize_chunk(c, ot)

            build_strip(0, xs_pre)
            attn_quarter(0)  # chunk c needs only strips <= c: each quarter
            build_strip(1, dma_strip(1))  # overlaps the next strip's build
            attn_quarter(1)
            build_strip(2, dma_strip(2))
            attn_quarter(2)
            build_strip(3, dma_strip(3))
            attn_quarter(3)

    nc.compile()
    return nc


_NC_CACHE = {}


def _get_nc():
    if "nc" not in _NC_CACHE:
        _NC_CACHE["nc"] = _build()
    return _NC_CACHE["nc"]


def make_in_maps(x, W_Q, W_K, W_V):
    x = np.ascontiguousarray(np.asarray(x, dtype=np.float32))
    W_Q = np.asarray(W_Q, dtype=np.float32)
    W_K = np.asarray(W_K, dtype=np.float32)
    W_V = np.asarray(W_V, dtype=np.float32)
    assert x.shape == (B, S, D)
    # weight layout prep (host, once): [j, d] -> packed d-major [d, j]
    wqk_host = np.ascontiguousarray(np.concatenate([W_Q.T, W_K.T], axis=1))
    wv_host = np.ascontiguousarray(W_V.T)
    return [
        {"x": np.ascontiguousarray(x[b]), "WQK": wqk_host, "WV": wv_host}
        for b in range(B)
    ]


def kernel(x, W_Q, W_K, W_V):
    nc = _get_nc()
    in_maps = make_in_maps(x, W_Q, W_K, W_V)
    res = run_bass_kernel_spmd(nc, in_maps, core_ids=list(range(B)))
    return np.stack([r["out"] for r in res.results], axis=0)


if __name__ == "__main__":
    rng = np.random.default_rng(0)
    inputs = {
        "x": rng.standard_normal((B, S, D), dtype=np.float32),
        "W_Q": (rng.random((J, D), dtype=np.float32) - 0.5) / 16.0,
        "W_K": (rng.random((J, D), dtype=np.float32) - 0.5) / 16.0,
        "W_V": (rng.random((J, D), dtype=np.float32) - 0.5) / 16.0,
    }
    got = kernel(**inputs)
    print("out", got.shape, got.dtype, np.abs(got).max())


# revision 19
# speedup vs baseline: 1.0488x; 1.0488x over previous
"""Trainium2 Bass kernel for a single-head causal attention module.

Problem (hardcoded): x [8, 2048, 1024] f32, W_Q/W_K/W_V [64, 1024] f32
    Q = x @ W_Q.T ; K = x @ W_K.T ; V = x @ W_V.T       (per batch)
    out = softmax(causal(Q @ K.T / sqrt(64))) @ V        -> [8, 2048, 64] f32

Sharding: batch dim across the 8 NeuronCores (data parallel, no collectives).

Per-core dataflow (all matmuls contract over the SBUF partition dim):
  1. PE-transpose x -> xT (d-major) since the QKV projections contract over d.
  2. One matmul chain per strip computes Q^T and K^T together (W_Q^T | W_K^T
     packed along the stationary free dim), output j-major which is exactly
     what the scores matmul needs to contract over j. Q^T/K^T strips are
     zero-padded to 128 partitions: a fp32 matmul with a 64-deep contraction
     streams at half rate, so padding the contraction doubles throughput.
  3. V^T likewise, then PE-transposed to V (s-major) with a ones column
     appended, so the P@V matmul also produces the softmax row-sums for free.
  4. Attention runs over 1024-wide q halves (pairs of 512 chunks): per key
     tile, scores land key-major (S^T) in a [128, 1024] PSUM half-strip, exp
     runs on ScalarE with the 1/sqrt(64) scale fused in (one call per half,
     amortizing ScalarE's fixed overhead), causal masking is an exact 0/1
     upper-triangular multiply on the diagonal block only (blocks above the
     diagonal are never computed), and P^T @ [V|1] accumulates O^T in PSUM.
  5. When a q chunk's last key tile retires, a PE transpose + reciprocal scale
     by the ones-column row-sum normalizes it into [128, 64] output tiles.
     The h=0 half (q < 1024) only needs strips 0-1, so it is emitted before
     strips 2-3 are built and overlaps their transposes/projections.
"""

import os

import numpy as np

import concourse.mybir as mybir
import concourse.tile as tile
from concourse import bacc
from concourse.bass_utils import run_bass_kernel_spmd
from concourse.masks import make_identity

B, S, D, J, P = 8, 2048, 1024, 64, 128
NCH = D // P  # 8 contraction chunks of 128
NSG = 4  # 512-wide s/q strips
SW = S // NSG  # 512
NT = S // P  # 16 key tiles
HW_ = 1024  # attention half-strip width
F32 = mybir.dt.float32

# Matmul input dtype: float32r streams 1 row/cycle (vs 4 for float32) with a
# 128-deep contraction; numerics measured at ~2e-4 max rel err end to end.
MM_DT = {
    "fp32": mybir.dt.float32,
    "fp32r": mybir.dt.float32r,
    "bf16": mybir.dt.bfloat16,
}[os.environ.get("ATTN_MM_DTYPE", "fp32r")]


def _build():
    nc = bacc.Bacc("TRN2", debug=False)
    x = nc.dram_tensor("x", [S, D], F32, kind="ExternalInput").ap()
    wqk = nc.dram_tensor("WQK", [D, P], F32, kind="ExternalInput").ap()
    wv = nc.dram_tensor("WV", [D, J], F32, kind="ExternalInput").ap()
    out = nc.dram_tensor("out", [S, J], F32, kind="ExternalOutput").ap()

    AF = mybir.ActivationFunctionType

    _fill = {}

    def zset(ap, val):  # memset that tolerates float32r tiles
        if ap.dtype != mybir.dt.float32r:
            nc.any.memset(ap, val)
            return
        # no engine memsets fp32r; copy-cast from a constant F32 tile instead
        src = _fill[val]
        nc.any.tensor_copy(ap, src[tuple(slice(0, d) for d in ap.shape)])

    with tile.TileContext(nc) as tc:
        from contextlib import ExitStack

        with ExitStack() as ctx:
            persist = ctx.enter_context(tc.tile_pool(name="persist", bufs=1))
            xsb_pool = ctx.enter_context(tc.tile_pool(name="xsb", bufs=2))
            wsb_pool = ctx.enter_context(tc.tile_pool(name="wsb", bufs=3))
            xsr_pool = ctx.enter_context(tc.tile_pool(name="xsr", bufs=2))
            pt_pool = ctx.enter_context(tc.tile_pool(name="ptp", bufs=3))
            otsb_pool = ctx.enter_context(tc.tile_pool(name="otsb", bufs=2))
            osb_pool = ctx.enter_context(tc.tile_pool(name="osb", bufs=3))
            rcp_pool = ctx.enter_context(tc.tile_pool(name="rcp", bufs=3))
            # PSUM budget (8 banks): ps512 x2 (projections, V/O transposes)
            # + sc1024 x2 (2 banks each: x transposes, scores) + 2 O^T accums.
            ps = ctx.enter_context(tc.tile_pool(name="ps", bufs=2, space="PSUM"))
            pssc = ctx.enter_context(tc.tile_pool(name="pssc", bufs=1, space="PSUM"))

            for val in (0.0, 1.0):
                ft = persist.tile([P, SW], F32, tag=f"fill{int(val)}", name=f"fill{int(val)}")
                nc.gpsimd.memset(ft, val)
                _fill[val] = ft

            ident = persist.tile([P, P], F32, tag="ident")
            make_identity(nc, ident)
            if MM_DT != F32:
                identm = persist.tile([P, P], MM_DT, tag="identm")
                nc.vector.tensor_copy(identm, ident)
            else:
                identm = ident
            # triu[p, f] = 1.0 iff f >= p  (valid: q_local >= k_local)
            triu = persist.tile([P, P], F32, tag="triu")
            nc.gpsimd.memset(triu, 1.0)
            nc.gpsimd.affine_select(
                out=triu,
                in_=triu,
                compare_op=mybir.AluOpType.is_ge,
                fill=0.0,
                base=0,
                pattern=[[1, P]],
                channel_multiplier=-1,
            )

            x_r = x.rearrange("(t p) d -> p t d", p=P)  # [128, 16, 1024]
            xs_pre = xsb_pool.tile([P, 4, D], F32, tag="xs", name="xs_pre")
            nc.sync.dma_start(xs_pre[:, 0:1, :], x_r[:, 0:1, :])
            nc.sync.dma_start(xs_pre[:, 1:2, :], x_r[:, 1:2, :])
            # W_Q^T | W_K^T packed along the stationary free dim (host
            # pre-transposes the tiny weights at model-load time); W_V^T alone.
            wqk_t = persist.tile([P, NCH, P], MM_DT, tag="wqkt")
            wv_t = persist.tile([P, NCH, J], MM_DT, tag="wvt")
            wqk_f = wsb_pool.tile([P, NCH, P], F32, tag="wqkf", name="wqk_f")
            wv_f = wsb_pool.tile([P, NCH, J], F32, tag="wvf", name="wv_f")
            nc.sync.dma_start(wqk_f, wqk.rearrange("(c p) m -> p c m", p=P))
            nc.sync.dma_start(wv_f, wv.rearrange("(c p) m -> p c m", p=P))
            if MM_DT != F32:
                nc.any.tensor_copy(wqk_t, wqk_f)
                nc.any.tensor_copy(wv_t, wv_f)
            else:
                wqk_t, wv_t = wqk_f, wv_f

            nc.sync.dma_start(xs_pre[:, 2:4, :], x_r[:, 2:4, :])

            xt_s = [
                persist.tile([P, NCH, SW], MM_DT, tag=f"xt{g}", name=f"xt{g}")
                for g in range(NSG)
            ]
            qt_s = [
                persist.tile([P, SW], MM_DT, tag=f"qt{g}", name=f"qt{g}")
                for g in range(NSG)
            ]
            kt_s = [
                persist.tile([P, SW], MM_DT, tag=f"kt{g}", name=f"kt{g}")
                for g in range(NSG)
            ]
            vaug_s = [
                persist.tile([P, 4, 72], MM_DT, tag=f"va{g}", name=f"va{g}")
                for g in range(NSG)
            ]

            out_r = out.rearrange("(t p) j -> p t j", p=P)  # [128, 16, 64]

            def dma_strip(sg):
                xs = xsb_pool.tile([P, 4, D], F32, tag="xs", name="xs")
                for half in range(2):  # split DMA so transposes start earlier
                    nc.sync.dma_start(
                        xs[:, 2 * half : 2 * half + 2, :],
                        x_r[:, 4 * sg + 2 * half : 4 * sg + 2 * half + 2, :],
                    )
                return xs

            def build_strip(sg, xs):
                """Transpose strip sg, project Q^T|K^T, V^T, build V|1."""
                for dch in range(NCH // 2):
                    pst = ps.tile([P, HW_], F32, tag="sc1024", name="pst")
                    for sub in range(2):
                        dc = 2 * dch + sub
                        for k in range(4):
                            nc.tensor.transpose(
                                pst[:, SW * sub + P * k : SW * sub + P * k + P],
                                xs[:, k, P * dc : P * dc + P],
                                ident,
                            )
                    nc.vector.tensor_copy(
                        xt_s[sg][:, 2 * dch : 2 * dch + 2, :],
                        pst.rearrange("p (c w) -> p c w", w=SW),
                    )
                zset(kt_s[sg][J:P, :], 0.0)
                psqk = ps.tile([P, SW], F32, tag="ps512", name="psqk")
                for dc in range(NCH):
                    nc.tensor.matmul(
                        psqk,
                        wqk_t[:, dc, :],
                        xt_s[sg][:, dc, :],
                        start=(dc == 0),
                        stop=(dc == NCH - 1),
                    )
                nc.any.tensor_copy(qt_s[sg][0:J, :], psqk[0:J])
                nc.any.tensor_copy(kt_s[sg][0:J, :], psqk[J:P])
                psv = ps.tile([P, SW], F32, tag="ps512", name="psv")
                for dc in range(NCH):
                    nc.tensor.matmul(
                        psv[0:J],
                        wv_t[:, dc, :],
                        xt_s[sg][:, dc, :],
                        start=(dc == 0),
                        stop=(dc == NCH - 1),
                    )
                # V^T parks in the (zero-weighted) bottom half of the q strip
                nc.any.tensor_copy(qt_s[sg][J:P, :], psv[0:J])
                nc.any.tensor_copy(
                    vaug_s[sg][:, :, J : J + 1], _fill[1.0][:, 0:4].unsqueeze(-1)
                )
                for k in range(4):
                    psv2 = ps.tile([P, 72], MM_DT, tag="ps512", name="psv2")
                    nc.tensor.transpose(
                        psv2[:, 0:J],
                        qt_s[sg][J:P, P * k : P * k + P],
                        identm[J:P, J:P] if MM_DT != F32 else ident[J:P, J:P],
                    )
                    nc.any.tensor_copy(vaug_s[sg][:, k, 0:J], psv2[:, 0:J])

            def finalize_chunk(c, ot):
                """Normalize O^T chunk c and write [128, 64] output tiles."""
                otsb = otsb_pool.tile([J + 1, SW], F32, tag="otsb", name="otsb")
                nc.any.tensor_copy(otsb, ot)
                # odd chunks: the other accumulator slot is free too, so the
                # four transposes double-buffer across both ot banks
                tags = ("ot0", "ot1") if c % 2 else (f"ot{c % 2}",)
                o = osb_pool.tile([P, 4, J], F32, tag="o", name="o")
                for k in range(4):
                    pso = pssc.tile([P, 72], F32, tag=tags[k % len(tags)], name="pso")
                    nc.tensor.transpose(
                        pso[:, 0 : J + 1],
                        otsb[:, P * k : P * k + P],
                        ident[0 : J + 1, 0 : J + 1],
                    )
                    rc = rcp_pool.tile([P, 1], F32, tag="rc", name="rc")
                    nc.vector.reciprocal(rc, pso[:, J : J + 1])
                    nc.vector.tensor_scalar_mul(out=o[:, k, :], in0=pso[:, 0:J], scalar1=rc)
                    if c == NSG - 1 and k % 2:  # drain the tail DMA early
                        nc.sync.dma_start(
                            out_r[:, 4 * c + k - 1 : 4 * c + k + 1, :],
                            o[:, k - 1 : k + 1, :],
                        )
                if c != NSG - 1:
                    nc.sync.dma_start(out_r[:, 4 * c : 4 * c + 4, :], o)

            def attn_half(h):
                """Scores/softmax/PV for q in [1024h, 1024h+1024)."""
                ot = {
                    c: pssc.tile([J + 1, SW], F32, tag=f"ot{c % 2}", name="ot")
                    for c in (2 * h, 2 * h + 1)
                }
                for t in range(8 * h + 8):
                    sgt, tl = t // 4, t % 4
                    off = max(0, P * t - HW_ * h)
                    chunks = range(max(2 * h, t // 4), 2 * h + 2)
                    pssh = ps.tile([P, HW_], F32, tag="sc1024", name="pssh")
                    for c in chunks:
                        lo = SW * c - HW_ * h
                        co = max(0, P * t - SW * c)
                        nc.tensor.matmul(
                            pssh[:, lo + co : lo + SW],
                            kt_s[sgt][:, P * tl : P * tl + P],
                            qt_s[c][:, co:SW],
                            start=True,
                            stop=True,
                        )
                    ptc = pt_pool.tile([P, HW_], MM_DT, tag="ptc", name="ptc")
                    nc.scalar.activation(
                        ptc[:, off:HW_], pssh[:, off:HW_], AF.Exp, scale=0.125
                    )
                    if P * t // HW_ == h:  # diagonal block lives in this half
                        nc.vector.tensor_mul(
                            ptc[:, off : off + P], ptc[:, off : off + P], triu
                        )
                    for c in chunks:
                        lo = SW * c - HW_ * h
                        co = max(0, P * t - SW * c)
                        nc.tensor.matmul(
                            ot[c][:, co:SW],
                            vaug_s[sgt][:, tl, 0 : J + 1],
                            ptc[:, lo + co : lo + SW],
                            start=(t == 0),
                            stop=(t == 4 * c + 3),
                        )
                        if t == 4 * c + 3:
                            finalize_chunk(c, ot[c])

            build_strip(0, xs_pre)
            build_strip(1, dma_strip(1))
            attn_half(0)  # only needs strips 0-1; overlaps strips 2-3 below
            build_strip(2, dma_strip(2))
            build_strip(3, dma_strip(3))
            attn_half(1)

    nc.compile()
    return nc


_NC_CACHE = {}


def _get_nc():
    if "nc" not in _NC_CACHE:
        _NC_CACHE["nc"] = _build()
    return _NC_CACHE["nc"]


def make_in_maps(x, W_Q, W_K, W_V):
    x = np.ascontiguousarray(np.asarray(x, dtype=np.float32))
    W_Q = np.asarray(W_Q, dtype=np.float32)
    W_K = np.asarray(W_K, dtype=np.float32)
    W_V = np.asarray(W_V, dtype=np.float32)
    assert x.shape == (B, S, D)
    # weight layout prep (host, once): [j, d] -> packed d-major [d, j]
    wqk_host = np.ascontiguousarray(np.concatenate([W_Q.T, W_K.T], axis=1))
    wv_host = np.ascontiguousarray(W_V.T)
    return [
        {"x": np.ascontiguousarray(x[b]), "WQK": wqk_host, "WV": wv_host}
        for b in range(B)
    ]


def kernel(x, W_Q, W_K, W_V):
    nc = _get_nc()
    in_maps = make_in_maps(x, W_Q, W_K, W_V)
    res = run_bass_kernel_spmd(nc, in_maps, core_ids=list(range(B)))
    return np.stack([r["out"] for r in res.results], axis=0)


if __name__ == "__main__":
    rng = np.random.default_rng(0)
    inputs = {
        "x": rng.standard_normal((B, S, D), dtype=np.float32),
        "W_Q": (rng.random((J, D), dtype=np.float32) - 0.5) / 16.0,
        "W_K": (rng.random((J, D), dtype=np.float32) - 0.5) / 16.0,
        "W_V": (rng.random((J, D), dtype=np.float32) - 0.5) / 16.0,
    }
    got = kernel(**inputs)
    print("out", got.shape, got.dtype, np.abs(got).max())
